# revision 9
# baseline (speedup 1.0000x reference)
"""Sparse-attention distance-mask kernel for Trainium2 (8 NeuronCores).

Reference computation (per batch b):
    pos      = multi-hot of 4 tree-position ids over 512 nodes   [seq, 512]
    dist     = s_i + s_j - 2 * pos @ pos.T          (L1 dist of binary vecs)
    attn     = max(dist_top, dist_left)
    out      = attn + padding_dist * max(pad_i, pad_j)

Kernel strategy:
  - Data-parallel over batch: core c computes batch c (b == n_cores == 8).
  - The whole distance-with-padding map folds into ONE augmented GEMM:
        dist + pad_mat = (-2 pos_i) . pos_j  +  A_i * 1  +  1 * B_j
                          + pad_i * (-p pad_j)
    with A_i = s_i + p*pad_i, B_j = s_j + p*pad_j, p = padding_dist.
    So lhsT = [-2 pos^T ; A ; 1 ; pad]  (K = 512+3), rhs = [pos^T ; 1 ; B ; -p*pad].
    Every operand/product is a small integer (|x| <= 256) -> exact in bf16.
  - On-device: 2 masks x 16 output tiles x 5 K-passes of [128,128]x[128,512]
    matmuls; phase A (top) copies PSUM->SBUF on DVE, phase B (left) does an
    in-place max against PSUM; 8 row-merged DMA stores.
  - Host does the (cheap) one-hot + augmentation prep in numpy.
"""

import os

import ml_dtypes
import numpy as np

B, SEQ, DEPTH = 8, 1024, 4
TN = 512          # TOTAL_NODE
KT = 5            # k-tiles: 4x128 pos rows + 1 aug tile (3 live rows)
AUG = 3
N_CORES = 8
MB, NB = SEQ // 128, SEQ // 512   # 8 x 2 output tiles of [128, 512]

_NC_CACHE = {}
LAST_RESULTS = None

_IN_NAMES = ("lhs_top", "rhs_top", "lhs_left", "rhs_left")


def _build_nc():
    import concourse.mybir as mybir
    from concourse import bacc
    from concourse.tile import TileContext

    nc = bacc.Bacc()
    dram = {}
    for name in _IN_NAMES:
        dram[name] = nc.dram_tensor(
            name, [128, KT * SEQ], mybir.dt.bfloat16, kind="ExternalInput"
        )
    out = nc.dram_tensor("out", [SEQ, SEQ], mybir.dt.float32, kind="ExternalOutput")

    with TileContext(nc) as tc:
        with (
            tc.tile_pool(name="w", bufs=1) as wpool,
            tc.tile_pool(name="ps", bufs=2, space="PSUM") as ppool,
            tc.tile_pool(name="ep", bufs=1) as epool,
        ):
            sb = {}
            for name in _IN_NAMES:
                w = wpool.tile([128, KT * SEQ], mybir.dt.bfloat16,
                               tag=name, name=name)
                nc.sync.dma_start(out=w[:, :], in_=dram[name][:, :])
                sb[name] = w

            cps = []
            for mb in range(MB):
                cp = epool.tile([128, SEQ], mybir.dt.float32,
                                tag=f"cp{mb}", name=f"cp{mb}")
                cps.append(cp)

            # Phase A: top-mask GEMMs -> copy into cp
            for mb in range(MB):
                for nb in range(NB):
                    ps_t = ppool.tile([128, 512], mybir.dt.float32, tag="pt",
                                      name=f"pt{mb}_{nb}")
                    for kt in range(KT):
                        ksz = 128 if kt < 4 else AUG
                        nc.tensor.matmul(
                            ps_t[:, :],
                            lhsT=sb["lhs_top"][0:ksz,
                                               kt * SEQ + mb * 128:
                                               kt * SEQ + mb * 128 + 128],
                            rhs=sb["rhs_top"][0:ksz,
                                              kt * SEQ + nb * 512:
                                              kt * SEQ + nb * 512 + 512],
                            start=(kt == 0),
                            stop=(kt == KT - 1),
                        )
                    nc.vector.tensor_copy(
                        cps[mb][:, nb * 512:(nb + 1) * 512], ps_t[:, :]
                    )

            # Phase B: left-mask GEMMs -> in-place max -> store
            for mb in range(MB):
                for nb in range(NB):
                    ps_l = ppool.tile([128, 512], mybir.dt.float32, tag="pl",
                                      name=f"pl{mb}_{nb}")
                    for kt in range(KT):
                        ksz = 128 if kt < 4 else AUG
                        nc.tensor.matmul(
                            ps_l[:, :],
                            lhsT=sb["lhs_left"][0:ksz,
                                                kt * SEQ + mb * 128:
                                                kt * SEQ + mb * 128 + 128],
                            rhs=sb["rhs_left"][0:ksz,
                                               kt * SEQ + nb * 512:
                                               kt * SEQ + nb * 512 + 512],
                            start=(kt == 0),
                            stop=(kt == KT - 1),
                        )
                    sl = cps[mb][:, nb * 512:(nb + 1) * 512]
                    nc.vector.tensor_max(sl, sl, ps_l[:, :])
                nc.sync.dma_start(out=out[mb * 128:(mb + 1) * 128, :],
                                  in_=cps[mb][:, :])
    nc.compile()
    return nc


def _host_prep(zipped, indicator, p):
    """Build augmented lhsT/rhs bf16 operands [B, 128, KT*SEQ] for one mask.

    Column block kt holds K-rows [kt*128, (kt+1)*128) of the augmented
    [515, seq] operand, laid out [128 partitions, seq]."""
    b, seq, depth = zipped.shape
    pos = np.zeros((b, seq, TN + 1), dtype=np.float32)
    np.put_along_axis(pos, np.asarray(zipped, dtype=np.int64), 1.0, axis=2)
    pos = pos[..., :TN]                       # [b, seq, 512] multi-hot
    s = pos.sum(axis=2)                       # [b, seq] distinct-id counts
    pad = (np.asarray(indicator) == 0).astype(np.float32)  # [b, seq]
    posT = pos.transpose(0, 2, 1)             # [b, 512, seq]

    lhs = np.zeros((b, KT, 128, seq), dtype=np.float32)
    rhs = np.zeros((b, KT, 128, seq), dtype=np.float32)
    lhs[:, :4] = (-2.0 * posT).reshape(b, 4, 128, seq)
    rhs[:, :4] = posT.reshape(b, 4, 128, seq)
    a_aug = s + p * pad
    lhs[:, 4, 0] = a_aug
    lhs[:, 4, 1] = 1.0
    lhs[:, 4, 2] = pad
    rhs[:, 4, 0] = 1.0
    rhs[:, 4, 1] = a_aug
    rhs[:, 4, 2] = -p * pad
    # [b, KT, 128, seq] -> [b, 128, KT*seq]
    lhs = lhs.transpose(0, 2, 1, 3).reshape(b, 128, KT * seq)
    rhs = rhs.transpose(0, 2, 1, 3).reshape(b, 128, KT * seq)
    return (np.ascontiguousarray(lhs).astype(ml_dtypes.bfloat16),
            np.ascontiguousarray(rhs).astype(ml_dtypes.bfloat16))


def kernel(zipped_top, zipped_left, indicator, padding_dist):
    global LAST_RESULTS
    from concourse.bass_utils import run_bass_kernel_spmd

    p = float(np.asarray(padding_dist))
    lhs_t, rhs_t = _host_prep(np.asarray(zipped_top), indicator, p)
    lhs_l, rhs_l = _host_prep(np.asarray(zipped_left), indicator, p)

    if "nc" not in _NC_CACHE:
        _NC_CACHE["nc"] = _build_nc()
    nc = _NC_CACHE["nc"]

    in_maps = [
        {
            "lhs_top": lhs_t[c], "rhs_top": rhs_t[c],
            "lhs_left": lhs_l[c], "rhs_left": rhs_l[c],
        }
        for c in range(N_CORES)
    ]
    res = run_bass_kernel_spmd(
        nc, in_maps, core_ids=list(range(N_CORES)),
        trace=os.environ.get("BASS_TRACE", "") == "1",
    )
    LAST_RESULTS = res
    return np.stack([res.results[c]["out"] for c in range(N_CORES)]).astype(
        np.float32
    )


# revision 10
# speedup vs baseline: 1.2297x; 1.2297x over previous
"""Sparse-attention distance-mask kernel for Trainium2 (8 NeuronCores).

Reference computation (per batch b):
    pos      = multi-hot of 4 tree-position ids over 512 nodes   [seq, 512]
    dist     = s_i + s_j - 2 * pos @ pos.T          (L1 dist of binary vecs)
    attn     = max(dist_top, dist_left)
    out      = attn + padding_dist * max(pad_i, pad_j)

Kernel strategy:
  - Data-parallel over batch: core c computes batch c (b == n_cores == 8).
  - The whole distance-with-padding map folds into ONE augmented GEMM:
        dist + pad_mat = (-2 pos_i) . pos_j  +  A_i * 1  +  1 * B_j
                          + pad_i * (-p pad_j)
    with A_i = s_i + p*pad_i, B_j = s_j + p*pad_j, p = padding_dist.
    lhsT = [-2 pos^T ; A ; 1 ; pad], rhs = [pos^T ; 1 ; B ; -p*pad], K = 515.
  - pos entries are {0, 1, -2} -> exact in fp8(e4m3); the pos block runs as
    2 DoubleRow fp8 matmul passes (K=256 each).  The 3 aug rows run as one
    bf16 K=3 pass (all values integers <= 256 -> bf16-exact for p=100).
    PSUM accumulates fp32, so the result is bit-exact vs the f32 reference.
  - The distance map is symmetric, so only 12 of 16 [128,512] output blocks
    are computed (blocks entirely below the diagonal are mirrored on host).
  - Epilogue per block: ACT copies top-PSUM to SBUF, DVE maxes left-PSUM in,
    row-merged DMA stores.
"""

import os

import ml_dtypes
import numpy as np

B, SEQ, DEPTH = 8, 1024, 4
TN = 512          # TOTAL_NODE
AUG = 3
N_CORES = 8
MB, NB = SEQ // 128, SEQ // 512   # 8 x 2 grid of [128, 512] blocks
# skip blocks entirely below the diagonal (mirrored on host)
BLOCKS = [(mb, nb) for mb in range(MB) for nb in range(NB)
          if not (nb == 0 and mb >= 4)]

_NC_CACHE = {}
LAST_RESULTS = None

_POS_NAMES = ("lhs_top", "rhs_top", "lhs_left", "rhs_left")


def _build_nc():
    import concourse.mybir as mybir
    from concourse import bacc
    from concourse.tile import TileContext

    nc = bacc.Bacc()
    dram = {}
    for name in _POS_NAMES:
        dram[name] = nc.dram_tensor(
            name, [128, 4 * SEQ], mybir.dt.float8e4, kind="ExternalInput"
        )
    dram["augs"] = nc.dram_tensor(
        "augs", [AUG, 4 * SEQ], mybir.dt.bfloat16, kind="ExternalInput"
    )
    out = nc.dram_tensor("out", [SEQ, SEQ], mybir.dt.float32, kind="ExternalOutput")

    with TileContext(nc) as tc:
        with (
            tc.tile_pool(name="w", bufs=1) as wpool,
            tc.tile_pool(name="ps", bufs=2, space="PSUM") as ppool,
            tc.tile_pool(name="ep", bufs=1) as epool,
        ):
            sb = {}
            # chunked loads, top-mask tensors first so PE can start early
            for name in _POS_NAMES:
                w = wpool.tile([128, 4 * SEQ], mybir.dt.float8e4,
                               tag=name, name=name)
                sb[name] = w
            augs = wpool.tile([AUG, 4 * SEQ], mybir.dt.bfloat16,
                              tag="augs", name="augs")
            for name in ("lhs_top", "rhs_top"):
                for c in range(2):
                    cs = slice(c * 2 * SEQ, (c + 1) * 2 * SEQ)
                    nc.sync.dma_start(out=sb[name][:, cs], in_=dram[name][:, cs])
            nc.sync.dma_start(out=augs[:, :], in_=dram["augs"][:, :])
            for name in ("lhs_left", "rhs_left"):
                for c in range(2):
                    cs = slice(c * 2 * SEQ, (c + 1) * 2 * SEQ)
                    nc.sync.dma_start(out=sb[name][:, cs], in_=dram[name][:, cs])

            # [128, 4*SEQ] -> [128, 4, SEQ] k-subtile views for DoubleRow
            v = {name: sb[name].rearrange("p (k s) -> p k s", s=SEQ)
                 for name in _POS_NAMES}

            cps = {}
            for mb in range(MB):
                wid = SEQ if mb < 4 else 512
                cps[mb] = epool.tile([128, wid], mybir.dt.float32,
                                     tag=f"cp{mb}", name=f"cp{mb}")

            def gemm(psum, lname, rname, aug_l, aug_r, mb, nb):
                ms = slice(mb * 128, (mb + 1) * 128)
                ns = slice(nb * 512, (nb + 1) * 512)
                for k2 in range(2):
                    nc.tensor.matmul(
                        psum[:, :],
                        lhsT=v[lname][:, 2 * k2:2 * k2 + 2, ms],
                        rhs=v[rname][:, 2 * k2:2 * k2 + 2, ns],
                        start=(k2 == 0),
                        stop=False,
                        perf_mode=mybir.MatmulPerfMode.DoubleRow,
                        skip_group_check=True,
                    )
                nc.tensor.matmul(
                    psum[:, :],
                    lhsT=augs[:, aug_l * SEQ + mb * 128:
                              aug_l * SEQ + mb * 128 + 128],
                    rhs=augs[:, aug_r * SEQ + nb * 512:
                             aug_r * SEQ + nb * 512 + 512],
                    start=False,
                    stop=True,
                    skip_group_check=True,
                )

            # Phase A: top-mask GEMMs -> ACT copy into cp
            for mb, nb in BLOCKS:
                ps_t = ppool.tile([128, 512], mybir.dt.float32, tag="pt",
                                  name=f"pt{mb}_{nb}")
                gemm(ps_t, "lhs_top", "rhs_top", 0, 1, mb, nb)
                off = nb * 512 if mb < 4 else 0
                nc.scalar.copy(cps[mb][:, off:off + 512], ps_t[:, :])

            # Phase B: left-mask GEMMs -> DVE max -> store
            done = set()
            for mb, nb in BLOCKS:
                ps_l = ppool.tile([128, 512], mybir.dt.float32, tag="pl",
                                  name=f"pl{mb}_{nb}")
                gemm(ps_l, "lhs_left", "rhs_left", 2, 3, mb, nb)
                off = nb * 512 if mb < 4 else 0
                sl = cps[mb][:, off:off + 512]
                nc.vector.tensor_max(sl, sl, ps_l[:, :])
                done.add((mb, nb))
                ms = slice(mb * 128, (mb + 1) * 128)
                if mb < 4 and (mb, 0) in done and (mb, 1) in done:
                    nc.sync.dma_start(out=out[ms, :], in_=cps[mb][:, :])
                elif mb >= 4:
                    nc.sync.dma_start(out=out[ms, 512:], in_=cps[mb][:, :])
    nc.compile()
    return nc


def _host_prep(zipped_top, zipped_left, indicator, p):
    """Build fp8 pos operands and the packed bf16 aug tensor."""
    fp8 = ml_dtypes.float8_e4m3
    bf16 = ml_dtypes.bfloat16
    pos = {}
    s = {}
    for key, zipped in (("top", zipped_top), ("left", zipped_left)):
        b, seq, depth = zipped.shape
        oh = np.zeros((b, seq, TN + 1), dtype=np.float32)
        np.put_along_axis(oh, np.asarray(zipped, dtype=np.int64), 1.0, axis=2)
        oh = oh[..., :TN]
        s[key] = oh.sum(axis=2)                       # [b, seq]
        # posT [b, 512, seq] -> [b, 128, 4*seq] with k-tile blocks along free
        posT = oh.transpose(0, 2, 1).reshape(b, 4, 128, seq)
        pos[key] = np.ascontiguousarray(posT.transpose(0, 2, 1, 3)
                                        ).reshape(b, 128, 4 * seq)
    pad = (np.asarray(indicator) == 0).astype(np.float32)  # [b, seq]
    b, seq = pad.shape

    ins = {
        "lhs_top": (-2.0 * pos["top"]).astype(fp8),
        "rhs_top": pos["top"].astype(fp8),
        "lhs_left": (-2.0 * pos["left"]).astype(fp8),
        "rhs_left": pos["left"].astype(fp8),
    }
    augs = np.zeros((b, AUG, 4 * seq), dtype=np.float32)
    for mi, key in enumerate(("top", "left")):
        a = s[key] + p * pad
        lo, ro = (2 * mi) * seq, (2 * mi + 1) * seq
        augs[:, 0, lo:lo + seq] = a          # lhs row 0: A_i
        augs[:, 1, lo:lo + seq] = 1.0        # lhs row 1: ones
        augs[:, 2, lo:lo + seq] = pad        # lhs row 2: pad_i
        augs[:, 0, ro:ro + seq] = 1.0        # rhs row 0: ones
        augs[:, 1, ro:ro + seq] = a          # rhs row 1: B_j
        augs[:, 2, ro:ro + seq] = -p * pad   # rhs row 2: -p*pad_j
    ins["augs"] = augs.astype(bf16)
    return ins


def kernel(zipped_top, zipped_left, indicator, padding_dist):
    global LAST_RESULTS
    from concourse.bass_utils import run_bass_kernel_spmd

    p = float(np.asarray(padding_dist))
    ins = _host_prep(np.asarray(zipped_top), np.asarray(zipped_left),
                     indicator, p)

    if "nc" not in _NC_CACHE:
        _NC_CACHE["nc"] = _build_nc()
    nc = _NC_CACHE["nc"]

    in_maps = [{k: v[c] for k, v in ins.items()} for c in range(N_CORES)]
    res = run_bass_kernel_spmd(
        nc, in_maps, core_ids=list(range(N_CORES)),
        trace=os.environ.get("BASS_TRACE", "") == "1",
    )
    LAST_RESULTS = res
    full = np.stack([res.results[c]["out"] for c in range(N_CORES)]).astype(
        np.float32
    )
    # mirror the skipped below-diagonal blocks: rows 512:1024, cols 0:512
    full[:, 512:, :512] = full[:, :512, 512:].transpose(0, 2, 1)
    return full
